# revision 45
# baseline (speedup 1.0000x reference)
"""AxialAttention Trainium2 Bass kernel (v2).

Problem: q,k,v of shape (4, 8, 16, 32, 32, 64) = (b, heads, t, h, w, d),
attention along the h axis (axis 3), softmax over keys, out same shape.

The computation is 512 independent "slabs" (b, heads, t), each a batch of
w=32 independent length-32 attention problems with head dim 64.  64 slabs
per NeuronCore (8 cores), processed in "quads" (4 slabs = 128 partitions).

Design notes (PE matmul cost ~ max(K_rows, N_cols) per instruction, so
weight loads and column streams are both minimized per problem):

  - Host pre-transposes Q and K to d-major layout, so no on-chip
    transposes are needed and every DMA is fully contiguous.
  - Scores: per (slab j, w) one K=64 matmul; the four slabs of a quad are
    packed as two "pair" tiles [128=(jj,d64), ...] and placed at PE
    quadrants (64*jj, 32*j), producing psS [128=(j,k), (w,q)] in PSUM.
  - exp on ScalarE over [128, 512] tiles (scale = 1/sqrt(64)).
  - Softmax denominator: one N=512 matmul per 16-w group with a constant
    block-diagonal ones matrix as weights: psD[(j,r),(w,q)] = sum_k
    E[(j,k),(w,q)].  Since every partition of a band carries the same
    value, RD = 1/psD (fast approx reciprocal) aligns with E
    partition-for-partition, and the normalize is fused into the
    block-diagonal scatter: E2bd[(j,k),(w,j,q)] = E * RD (DVE).
  - PV: ONE matmul per w with the 128x128 block-diagonal E2bd as weights
    and rhs=V natural [128=(j,k), d] -> psPV [128=(j,q), d]; one weight
    load covers all four slabs.
  - psPV copied (and cast) to bf16 out_sb on ScalarE, stored
    contiguously; host casts back to fp32.
  - Software pipeline: PV lags scores by 3 groups, denominators by 1;
    loads prefetch 3 quads ahead; PV/copies are emitted before scores so
    the Scalar queue frees PSUM before blocking on the next exp.
"""

import os
import sys
import numpy as np

for _p in ("/root/.axon_site/_ro/trn_rl_repo", "/opt/trn_rl_repo"):
    if os.path.isdir(_p) and _p not in sys.path:
        sys.path.append(_p)

B, NH, T, H, W, D = 4, 8, 16, 32, 32, 64
N_CORES = 8
NSLAB = B * NH * T  # 512
NSLAB_CORE = NSLAB // N_CORES  # 64
NQUAD = NSLAB_CORE // 4  # 16
NGRP = 2 * NQUAD  # 16-w score/exp/pv groups per core

_CACHED_NC = None


def _build_nc():
    import concourse.bacc as bacc
    import concourse.mybir as mybir
    from concourse import tile

    dt = mybir.dt

    nc = bacc.Bacc("TRN2", target_bir_lowering=False, debug=False,
                   num_devices=N_CORES)
    # (quad, pair, (jj,d64), (w,q))
    qt_in = nc.dram_tensor("qt_in", [NQUAD, 2, 128, W * 32], dt.bfloat16,
                           kind="ExternalInput").ap()
    kt_in = nc.dram_tensor("kt_in", [NQUAD, 2, 128, W * 32], dt.bfloat16,
                           kind="ExternalInput").ap()
    # (quad, (j,k=h), (w,d))
    v_in = nc.dram_tensor("v_in", [NQUAD, 128, W * D], dt.bfloat16,
                          kind="ExternalInput").ap()
    # (quad, (j,q=h), (w,d))
    o_out = nc.dram_tensor("o_out", [NQUAD, 128, W * D], dt.bfloat16,
                           kind="ExternalOutput").ap()

    scale = 1.0 / float(np.sqrt(D))

    with tile.TileContext(nc) as tc:
        with tc.tile_pool(name="io", bufs=6) as io_pool, \
             tc.tile_pool(name="vv", bufs=6) as v_pool, \
             tc.tile_pool(name="ee", bufs=4) as e_pool, \
             tc.tile_pool(name="e2", bufs=5) as e2_pool, \
             tc.tile_pool(name="rr", bufs=3) as r_pool, \
             tc.tile_pool(name="oo", bufs=5) as o_pool, \
             tc.tile_pool(name="cs", bufs=1) as c_pool, \
             tc.tile_pool(name="ps_sc", bufs=3, space="PSUM") as ps_sc, \
             tc.tile_pool(name="ps_d", bufs=2, space="PSUM") as ps_d, \
             tc.tile_pool(name="ps_pv", bufs=3, space="PSUM") as ps_pv:

            # Constant block-diagonal ones [128=(j,k), 128=(j,r)] used to
            # compute softmax denominators via one matmul per group.
            ones_bd = c_pool.tile([128, 128], dt.bfloat16, name="ones_bd")
            nc.vector.memset(ones_bd[:, :], 0.0)
            for j in range(4):
                nc.vector.memset(
                    ones_bd[32 * j:32 * j + 32, 32 * j:32 * j + 32], 1.0)

            qstate = {}
            gstate = {}

            def emit_loads(g):
                QT = io_pool.tile([128, 2, W * 32], dt.bfloat16, name="QT")
                KT = io_pool.tile([128, 2, W * 32], dt.bfloat16, name="KT")
                V4 = v_pool.tile([128, W * D], dt.bfloat16, name="V4")
                for p in range(2):
                    nc.sync.dma_start(out=QT[:, p, :], in_=qt_in[g, p])
                    nc.sync.dma_start(out=KT[:, p, :], in_=kt_in[g, p])
                nc.gpsimd.dma_start(out=V4[:, :], in_=v_in[g])
                out_sb = o_pool.tile([128, W * D], dt.bfloat16,
                                     name="out_sb")
                qstate[g] = dict(QT=QT, KT=KT, V4=V4, out_sb=out_sb)

            def emit_scores(i):
                g, grp = divmod(i, 2)
                qs = qstate[g]
                QT, KT = qs["QT"], qs["KT"]
                psS = ps_sc.tile([128, 16, 32], dt.float32, name="psS")
                for wl in range(16):
                    w = 16 * grp + wl
                    for j in range(4):
                        p, jj = divmod(j, 2)
                        nc.tensor.matmul(
                            psS[32 * j:32 * j + 32, wl, :],
                            lhsT=KT[64 * jj:64 * jj + 64, p,
                                    32 * w:32 * w + 32],
                            rhs=QT[64 * jj:64 * jj + 64, p,
                                   32 * w:32 * w + 32],
                            start=True, stop=True,
                            tile_position=(64 * jj, 32 * j))
                E = e_pool.tile([128, 16, 32], dt.bfloat16, name="E")
                nc.scalar.activation(
                    E[:, :, :], psS[:, :, :],
                    mybir.ActivationFunctionType.Exp, scale=scale)
                gstate[i] = dict(E=E)

            e2bd_allocs = [0]

            def emit_denom(i):
                gs = gstate[i]
                E = gs["E"]
                psD = ps_d.tile([128, 16, 32], dt.float32, name="psD")
                nc.tensor.matmul(
                    psD[:, :, :], lhsT=ones_bd[:, :], rhs=E[:, :, :],
                    start=True, stop=True)
                RD = r_pool.tile([128, 16, 32], dt.float32, name="RD")
                nc.vector.reciprocal_approx_fast(out=RD[:, :, :],
                                                 in_=psD[:, :, :])
                RDb = r_pool.tile([128, 16, 32], dt.bfloat16, name="RDb")
                nc.vector.tensor_copy(RDb[:, :, :], RD[:, :, :])
                # Normalized E scattered into a block-diagonal weight tile
                # [128=(j,k), (w, j', q)]; off-diagonal blocks stay zero
                # (zeroed once per pool buffer).
                E2bd = e2_pool.tile([128, 16, 4, 32], dt.bfloat16,
                                    name="E2bd")
                if e2bd_allocs[0] < 5:
                    e2bd_allocs[0] += 1
                    nc.vector.memset(E2bd[:, :, :, :], 0.0)
                for j in range(4):
                    nc.vector.tensor_mul(
                        E2bd[32 * j:32 * j + 32, :, j, :],
                        E[32 * j:32 * j + 32, :, :],
                        RDb[32 * j:32 * j + 32, :, :])
                gs["E2bd"] = E2bd

            def emit_pv(i):
                g, grp = divmod(i, 2)
                qs = qstate[g]
                gs = gstate.pop(i)
                V4, out_sb = qs["V4"], qs["out_sb"]
                E2bd = gs["E2bd"]
                for half in range(2):
                    psPV = ps_pv.tile([128, 8 * D], dt.float32, name="psPV")
                    for wl8 in range(8):
                        wl = 8 * half + wl8
                        w = 16 * grp + wl
                        nc.tensor.matmul(
                            psPV[:, D * wl8:D * wl8 + D],
                            lhsT=E2bd[:, wl, :, :],
                            rhs=V4[:, D * w:D * w + D],
                            start=True, stop=True)
                    w0 = 16 * grp + 8 * half
                    nc.scalar.copy(out_sb[:, D * w0:D * w0 + 8 * D],
                                   psPV[:, :])
                if grp == 1:
                    nc.gpsimd.dma_start(out=o_out[g], in_=out_sb[:, :])
                    qstate.pop(g)

            # Software pipeline with a 2-group lag so the PE queue always
            # has runnable matmuls while exp / recip / normalize drain on
            # the Scalar and Vector engines.  PV/copies are emitted before
            # scores/exp each round so the Scalar queue frees PSUM buffers
            # before blocking on the next exp's input.
            emit_loads(0)
            emit_loads(1)
            emit_loads(2)
            for i in range(NGRP):
                g, grp = divmod(i, 2)
                if grp == 0 and g + 3 < NQUAD:
                    emit_loads(g + 3)
                if i >= 3:
                    emit_pv(i - 3)
                emit_scores(i)
                if i >= 1:
                    emit_denom(i - 1)
            emit_denom(NGRP - 1)
            emit_pv(NGRP - 3)
            emit_pv(NGRP - 2)
            emit_pv(NGRP - 1)
    nc.compile()
    return nc


def _get_nc():
    global _CACHED_NC
    if _CACHED_NC is None:
        _CACHED_NC = _build_nc()
    return _CACHED_NC


def kernel(q, k, v, decode_step=0, decode_idx=0, _trace=False):
    from concourse.bass_utils import run_bass_kernel_spmd

    import ml_dtypes
    bf16 = ml_dtypes.bfloat16
    q = np.asarray(q, dtype=np.float32).reshape(NSLAB, H, W, D).astype(bf16)
    k = np.asarray(k, dtype=np.float32).reshape(NSLAB, H, W, D).astype(bf16)
    v = np.asarray(v, dtype=np.float32).reshape(NSLAB, H, W, D).astype(bf16)

    # d-major transpose for Q/K: (slab, d, w, h); V stays natural.
    qt = np.ascontiguousarray(q.transpose(0, 3, 2, 1))
    kt = np.ascontiguousarray(k.transpose(0, 3, 2, 1))

    nc = _get_nc()
    in_maps = []
    for c in range(N_CORES):
        sl = slice(c * NSLAB_CORE, (c + 1) * NSLAB_CORE)
        # (64, 64, 32, 32) -> (quad, pair, (jj,d), (w,q)) -> [16, 2, 128, 1024]
        qtc = qt[sl].reshape(NQUAD, 2, 2 * D, W * 32)
        ktc = kt[sl].reshape(NQUAD, 2, 2 * D, W * 32)
        vc = v[sl].reshape(NQUAD, 128, W * D)
        in_maps.append({
            "qt_in": np.ascontiguousarray(qtc),
            "kt_in": np.ascontiguousarray(ktc),
            "v_in": np.ascontiguousarray(vc),
        })
    res = run_bass_kernel_spmd(nc, in_maps, core_ids=list(range(N_CORES)),
                               trace=_trace)
    outs = []
    for r in res.results:
        # [16, 128, 2048] = (quad, (j, h), (w, d)) -> (slab, h, w, d)
        o = np.asarray(r["o_out"]).reshape(NSLAB_CORE, H, W, D)
        outs.append(o)
    out = np.concatenate(outs, axis=0).astype(np.float32)
    out = out.reshape(B, NH, T, H, W, D)
    if _trace:
        return out, res
    return out


if __name__ == "__main__":
    rng = np.random.default_rng(0)
    shape = (B, NH, T, H, W, D)
    q = rng.standard_normal(shape, dtype=np.float32)
    k = rng.standard_normal(shape, dtype=np.float32)
    v = rng.standard_normal(shape, dtype=np.float32)
    out = kernel(q, k, v)
    print("kernel ran, out shape", out.shape)


# revision 46
# speedup vs baseline: 1.0348x; 1.0348x over previous
"""AxialAttention Trainium2 Bass kernel (v2).

Problem: q,k,v of shape (4, 8, 16, 32, 32, 64) = (b, heads, t, h, w, d),
attention along the h axis (axis 3), softmax over keys, out same shape.

The computation is 512 independent "slabs" (b, heads, t), each a batch of
w=32 independent length-32 attention problems with head dim 64.  64 slabs
per NeuronCore (8 cores), processed in "quads" (4 slabs = 128 partitions).

Design notes (PE matmul cost ~ max(K_rows, N_cols) per instruction, so
weight loads and column streams are both minimized per problem):

  - Host pre-transposes Q and K to d-major layout, so no on-chip
    transposes are needed and every DMA is fully contiguous.
  - Scores: per (slab j, w) one K=64 matmul; the four slabs of a quad are
    packed as two "pair" tiles [128=(jj,d64), ...] and placed at PE
    quadrants (64*jj, 32*j), producing psS [128=(j,k), (w,q)] in PSUM.
  - exp on ScalarE over [128, 512] tiles (scale = 1/sqrt(64)).
  - Softmax denominator: one N=512 matmul per 16-w group with a constant
    block-diagonal ones matrix as weights: psD[(j,r),(w,q)] = sum_k
    E[(j,k),(w,q)].  Since every partition of a band carries the same
    value, RD = 1/psD (fast approx reciprocal) aligns with E
    partition-for-partition, and the normalize is fused into the
    block-diagonal scatter: E2bd[(j,k),(w,j,q)] = E * RD (DVE).
  - PV: ONE matmul per w with the 128x128 block-diagonal E2bd as weights
    and rhs=V natural [128=(j,k), d] -> psPV [128=(j,q), d]; one weight
    load covers all four slabs.
  - psPV copied (and cast) to bf16 out_sb on ScalarE, stored
    contiguously; host casts back to fp32.
  - Software pipeline: PV lags scores by 3 groups, denominators by 1;
    loads prefetch 3 quads ahead; PV/copies are emitted before scores so
    the Scalar queue frees PSUM before blocking on the next exp.
"""

import os
import sys
import numpy as np

for _p in ("/root/.axon_site/_ro/trn_rl_repo", "/opt/trn_rl_repo"):
    if os.path.isdir(_p) and _p not in sys.path:
        sys.path.append(_p)

B, NH, T, H, W, D = 4, 8, 16, 32, 32, 64
N_CORES = 8
NSLAB = B * NH * T  # 512
NSLAB_CORE = NSLAB // N_CORES  # 64
NQUAD = NSLAB_CORE // 4  # 16
NGRP = 2 * NQUAD  # 16-w score/exp/pv groups per core

_CACHED_NC = None


def _build_nc():
    import concourse.bacc as bacc
    import concourse.mybir as mybir
    from concourse import tile

    dt = mybir.dt

    nc = bacc.Bacc("TRN2", target_bir_lowering=False, debug=False,
                   num_devices=N_CORES)
    # (quad, pair, (jj,d64), (w,q))
    qt_in = nc.dram_tensor("qt_in", [NQUAD, 2, 128, W * 32], dt.bfloat16,
                           kind="ExternalInput").ap()
    kt_in = nc.dram_tensor("kt_in", [NQUAD, 2, 128, W * 32], dt.bfloat16,
                           kind="ExternalInput").ap()
    # (quad, (j,k=h), (w,d))
    v_in = nc.dram_tensor("v_in", [NQUAD, 128, W * D], dt.bfloat16,
                          kind="ExternalInput").ap()
    # (quad, (j,q=h), (w,d))
    o_out = nc.dram_tensor("o_out", [NQUAD, 128, W * D], dt.bfloat16,
                           kind="ExternalOutput").ap()

    scale = 1.0 / float(np.sqrt(D))

    with tile.TileContext(nc) as tc:
        with tc.tile_pool(name="io", bufs=6) as io_pool, \
             tc.tile_pool(name="vv", bufs=6) as v_pool, \
             tc.tile_pool(name="ee", bufs=4) as e_pool, \
             tc.tile_pool(name="e2", bufs=5) as e2_pool, \
             tc.tile_pool(name="rr", bufs=3) as r_pool, \
             tc.tile_pool(name="oo", bufs=5) as o_pool, \
             tc.tile_pool(name="cs", bufs=1) as c_pool, \
             tc.tile_pool(name="ps_sc", bufs=2, space="PSUM") as ps_sc, \
             tc.tile_pool(name="ps_d", bufs=3, space="PSUM") as ps_d, \
             tc.tile_pool(name="ps_pv", bufs=3, space="PSUM") as ps_pv:

            # Constant block-diagonal ones [128=(j,k), 128=(j,r)] used to
            # compute softmax denominators via one matmul per group.
            ones_bd = c_pool.tile([128, 128], dt.bfloat16, name="ones_bd")
            nc.vector.memset(ones_bd[:, :], 0.0)
            for j in range(4):
                nc.vector.memset(
                    ones_bd[32 * j:32 * j + 32, 32 * j:32 * j + 32], 1.0)

            qstate = {}
            gstate = {}

            def emit_loads(g):
                QT = io_pool.tile([128, 2, W * 32], dt.bfloat16, name="QT")
                KT = io_pool.tile([128, 2, W * 32], dt.bfloat16, name="KT")
                V4 = v_pool.tile([128, W * D], dt.bfloat16, name="V4")
                for p in range(2):
                    nc.sync.dma_start(out=QT[:, p, :], in_=qt_in[g, p])
                    nc.sync.dma_start(out=KT[:, p, :], in_=kt_in[g, p])
                nc.gpsimd.dma_start(out=V4[:, :], in_=v_in[g])
                out_sb = o_pool.tile([128, W * D], dt.bfloat16,
                                     name="out_sb")
                qstate[g] = dict(QT=QT, KT=KT, V4=V4, out_sb=out_sb)

            def emit_scores(i, pv_i=None):
                g, grp = divmod(i, 2)
                qs = qstate[g]
                QT, KT = qs["QT"], qs["KT"]
                pv = _pv_parts(pv_i) if pv_i is not None else None
                psS = ps_sc.tile([128, 16, 32], dt.float32, name="psS")
                for wl in range(16):
                    w = 16 * grp + wl
                    for j in range(4):
                        p, jj = divmod(j, 2)
                        nc.tensor.matmul(
                            psS[32 * j:32 * j + 32, wl, :],
                            lhsT=KT[64 * jj:64 * jj + 64, p,
                                    32 * w:32 * w + 32],
                            rhs=QT[64 * jj:64 * jj + 64, p,
                                   32 * w:32 * w + 32],
                            start=True, stop=True,
                            tile_position=(64 * jj, 32 * j))
                    if pv is not None:
                        pv(wl)
                E = e_pool.tile([128, 16, 32], dt.bfloat16, name="E")
                nc.scalar.activation(
                    E[:, :, :], psS[:, :, :],
                    mybir.ActivationFunctionType.Exp, scale=scale)
                gstate[i] = dict(E=E)

            e2bd_allocs = [0]

            def emit_denom(i):
                gs = gstate[i]
                E = gs["E"]
                psD = ps_d.tile([128, 16, 32], dt.float32, name="psD")
                nc.tensor.matmul(
                    psD[:, :, :], lhsT=ones_bd[:, :], rhs=E[:, :, :],
                    start=True, stop=True)
                RD = r_pool.tile([128, 16, 32], dt.float32, name="RD")
                nc.vector.reciprocal_approx_fast(out=RD[:, :, :],
                                                 in_=psD[:, :, :])
                RDb = r_pool.tile([128, 16, 32], dt.bfloat16, name="RDb")
                nc.vector.tensor_copy(RDb[:, :, :], RD[:, :, :])
                # Normalized E scattered into a block-diagonal weight tile
                # [128=(j,k), (w, j', q)]; off-diagonal blocks stay zero
                # (zeroed once per pool buffer).
                E2bd = e2_pool.tile([128, 16, 4, 32], dt.bfloat16,
                                    name="E2bd")
                if e2bd_allocs[0] < 5:
                    e2bd_allocs[0] += 1
                    nc.vector.memset(E2bd[:, :, :, :], 0.0)
                for j in range(4):
                    nc.vector.tensor_mul(
                        E2bd[32 * j:32 * j + 32, :, j, :],
                        E[32 * j:32 * j + 32, :, :],
                        RDb[32 * j:32 * j + 32, :, :])
                gs["E2bd"] = E2bd

            def _pv_parts(i):
                g, grp = divmod(i, 2)
                qs = qstate[g]
                gs = gstate.pop(i)
                V4, out_sb = qs["V4"], qs["out_sb"]
                E2bd = gs["E2bd"]
                state = {}

                def step(wl):
                    half, wl8 = divmod(wl, 8)
                    if wl8 == 0:
                        state["psPV"] = ps_pv.tile([128, 8 * D],
                                                   dt.float32, name="psPV")
                    psPV = state["psPV"]
                    w = 16 * grp + wl
                    nc.tensor.matmul(
                        psPV[:, D * wl8:D * wl8 + D],
                        lhsT=E2bd[:, wl, :, :],
                        rhs=V4[:, D * w:D * w + D],
                        start=True, stop=True)
                    if wl8 == 7:
                        w0 = 16 * grp + 8 * half
                        nc.scalar.copy(
                            out_sb[:, D * w0:D * w0 + 8 * D], psPV[:, :])
                    if wl == 15 and grp == 1:
                        nc.gpsimd.dma_start(out=o_out[g], in_=out_sb[:, :])
                        qstate.pop(g)
                return step

            def emit_pv(i):
                pv = _pv_parts(i)
                for wl in range(16):
                    pv(wl)

            # Software pipeline with a 2-group lag so the PE queue always
            # has runnable matmuls while exp / recip / normalize drain on
            # the Scalar and Vector engines.  PV/copies are emitted before
            # scores/exp each round so the Scalar queue frees PSUM buffers
            # before blocking on the next exp's input.
            emit_loads(0)
            emit_loads(1)
            emit_loads(2)
            for i in range(NGRP):
                g, grp = divmod(i, 2)
                if grp == 0 and g + 3 < NQUAD:
                    emit_loads(g + 3)
                emit_scores(i, pv_i=(i - 3) if i >= 3 else None)
                if i >= 1:
                    emit_denom(i - 1)
            emit_denom(NGRP - 1)
            emit_pv(NGRP - 3)
            emit_pv(NGRP - 2)
            emit_pv(NGRP - 1)
    nc.compile()
    return nc


def _get_nc():
    global _CACHED_NC
    if _CACHED_NC is None:
        _CACHED_NC = _build_nc()
    return _CACHED_NC


def kernel(q, k, v, decode_step=0, decode_idx=0, _trace=False):
    from concourse.bass_utils import run_bass_kernel_spmd

    import ml_dtypes
    bf16 = ml_dtypes.bfloat16
    q = np.asarray(q, dtype=np.float32).reshape(NSLAB, H, W, D).astype(bf16)
    k = np.asarray(k, dtype=np.float32).reshape(NSLAB, H, W, D).astype(bf16)
    v = np.asarray(v, dtype=np.float32).reshape(NSLAB, H, W, D).astype(bf16)

    # d-major transpose for Q/K: (slab, d, w, h); V stays natural.
    qt = np.ascontiguousarray(q.transpose(0, 3, 2, 1))
    kt = np.ascontiguousarray(k.transpose(0, 3, 2, 1))

    nc = _get_nc()
    in_maps = []
    for c in range(N_CORES):
        sl = slice(c * NSLAB_CORE, (c + 1) * NSLAB_CORE)
        # (64, 64, 32, 32) -> (quad, pair, (jj,d), (w,q)) -> [16, 2, 128, 1024]
        qtc = qt[sl].reshape(NQUAD, 2, 2 * D, W * 32)
        ktc = kt[sl].reshape(NQUAD, 2, 2 * D, W * 32)
        vc = v[sl].reshape(NQUAD, 128, W * D)
        in_maps.append({
            "qt_in": np.ascontiguousarray(qtc),
            "kt_in": np.ascontiguousarray(ktc),
            "v_in": np.ascontiguousarray(vc),
        })
    res = run_bass_kernel_spmd(nc, in_maps, core_ids=list(range(N_CORES)),
                               trace=_trace)
    outs = []
    for r in res.results:
        # [16, 128, 2048] = (quad, (j, h), (w, d)) -> (slab, h, w, d)
        o = np.asarray(r["o_out"]).reshape(NSLAB_CORE, H, W, D)
        outs.append(o)
    out = np.concatenate(outs, axis=0).astype(np.float32)
    out = out.reshape(B, NH, T, H, W, D)
    if _trace:
        return out, res
    return out


if __name__ == "__main__":
    rng = np.random.default_rng(0)
    shape = (B, NH, T, H, W, D)
    q = rng.standard_normal(shape, dtype=np.float32)
    k = rng.standard_normal(shape, dtype=np.float32)
    v = rng.standard_normal(shape, dtype=np.float32)
    out = kernel(q, k, v)
    print("kernel ran, out shape", out.shape)
